# revision 5
# baseline (speedup 1.0000x reference)
"""MoE layer (8 experts, top-2, SwiGLU) on 8 TRN2 NeuronCores.

Strategy: expert-parallel. The router (x @ Wr, top-2, softmax) runs on the
host — it is ~0.03% of the FLOPs. Tokens are dispatched per expert on the
host (the "all-to-all"), each core runs its expert's dense SwiGLU MLP over
its (capacity-padded) token batch in fp32r, and the host applies the
combine weights (including DEPTH_SCALE) on the way back.

Device layout (per core / expert e):
  xt   [C, CAP]          = gathered tokens, transposed (feature-major)
  wg_t [H/128,128,C/128,128]  Wg[e] pre-tiled so each lhsT tile DMA is contiguous
  wu_t same
  wd_t [C/128,128,H/128,128]  Wd[e] pre-tiled
  yt   [C, CAP]          = (silu(x@Wg) * (x@Wu)) @ Wd, transposed, unscaled

All matmuls are fp32r (full PE rate at free-dim >= 256, ~1e-4 rounding).
The hidden dimension is processed in halves so h fits in SBUF; the second
half accumulates into yt with an accumulating DMA.
"""

import sys

if "/opt/trn_rl_repo" not in sys.path:
    sys.path.insert(0, "/opt/trn_rl_repo")

import numpy as np

D_MODEL = 1024
HIDDEN = 4096
NUM_EXPERTS = 8
TOP_K = 2
DEPTH_SCALE = 1.0 / np.sqrt(12.0)

P = 128
NC = D_MODEL // P     # 8 column chunks of the model dim
NH = HIDDEN // P      # 32 chunks of the hidden dim
TS = 384              # token sub-tile (>=256 keeps fp32r at full PE rate)
H_SPLIT = 2           # hidden halves, bounds h to 16*CAP*4 bytes/partition


def _build_nc(cap):
    import concourse.bass as bass
    import concourse.mybir as mybir
    import concourse.tile as tile
    from concourse import bacc

    F32 = mybir.dt.float32
    F32R = mybir.dt.float32r
    nt = cap // TS
    nh_half = NH // H_SPLIT

    nc = bacc.Bacc("TRN2", target_bir_lowering=False, debug=False, num_devices=8)
    xt = nc.dram_tensor("xt", [D_MODEL, cap], F32R, kind="ExternalInput").ap()
    wg_t = nc.dram_tensor("wg_t", [NH, P, NC, P], F32R, kind="ExternalInput").ap()
    wu_t = nc.dram_tensor("wu_t", [NH, P, NC, P], F32R, kind="ExternalInput").ap()
    wd_t = nc.dram_tensor("wd_t", [NC, P, NH, P], F32R, kind="ExternalInput").ap()
    yt = [
        nc.dram_tensor(f"yt{i}", [D_MODEL, cap], F32, kind="ExternalOutput").ap()
        for i in range(H_SPLIT)
    ]

    with tile.TileContext(nc) as tc:
        with (
            tc.tile_pool(name="xpool", bufs=1) as xpool,
            tc.tile_pool(name="hpool", bufs=1) as hpool,
            tc.tile_pool(name="wg", bufs=3) as wgp,
            tc.tile_pool(name="wu", bufs=3) as wup,
            tc.tile_pool(name="wd", bufs=2) as wdp,
            tc.tile_pool(name="ypool", bufs=2) as ypool,
            tc.tile_pool(name="psum", bufs=8, space="PSUM") as psp,
        ):
            xt_sb = xpool.tile([P, NC, cap], F32R)
            nc.sync.dma_start(xt_sb[:], xt.rearrange("(o p) n -> p o n", p=P))

            for half in range(H_SPLIT):
                h_sb = hpool.tile([P, nh_half, cap], F32R, tag="h")
                for hh in range(nh_half):
                    hc = half * nh_half + hh
                    wg_sb = wgp.tile([P, NC, P], F32R, tag="wg")
                    nc.sync.dma_start(wg_sb[:], wg_t[hc])
                    wu_sb = wup.tile([P, NC, P], F32R, tag="wu")
                    nc.sync.dma_start(wu_sb[:], wu_t[hc])

                    pg = [psp.tile([P, TS], F32, tag="ps", name=f"pg{hc}_{t}") for t in range(nt)]
                    pu = [psp.tile([P, TS], F32, tag="ps", name=f"pu{hc}_{t}") for t in range(nt)]
                    for c in range(NC):
                        for t in range(nt):
                            nc.tensor.matmul(
                                pg[t][:],
                                wg_sb[:, c],
                                xt_sb[:, c, t * TS:(t + 1) * TS],
                                start=(c == 0),
                                stop=(c == NC - 1),
                            )
                    for c in range(NC):
                        for t in range(nt):
                            nc.tensor.matmul(
                                pu[t][:],
                                wu_sb[:, c],
                                xt_sb[:, c, t * TS:(t + 1) * TS],
                                start=(c == 0),
                                stop=(c == NC - 1),
                            )
                    for t in range(nt):
                        hseg = h_sb[:, hh, t * TS:(t + 1) * TS]
                        nc.scalar.activation(
                            hseg, pg[t][:], mybir.ActivationFunctionType.Silu
                        )
                        nc.vector.tensor_mul(hseg, hseg, pu[t][:])

                for oc in range(NC):
                    wd_sb = wdp.tile([P, nh_half, P], F32R, tag="wd")
                    nc.sync.dma_start(
                        wd_sb[:], wd_t[oc, :, half * nh_half:(half + 1) * nh_half]
                    )
                    py = [psp.tile([P, TS], F32, tag="ps", name=f"py{half}_{oc}_{t}") for t in range(nt)]
                    for hh in range(nh_half):
                        for t in range(nt):
                            nc.tensor.matmul(
                                py[t][:],
                                wd_sb[:, hh],
                                h_sb[:, hh, t * TS:(t + 1) * TS],
                                start=(hh == 0),
                                stop=(hh == nh_half - 1),
                            )
                    y_sb = ypool.tile([P, cap], F32, tag="y")
                    for t in range(nt):
                        nc.vector.tensor_copy(
                            y_sb[:, t * TS:(t + 1) * TS], py[t][:]
                        )
                    nc.sync.dma_start(yt[half][oc * P:(oc + 1) * P, :], y_sb[:])

    nc.compile()
    return nc


def _route(flat_x, Wr):
    """Host router: returns per-expert (token_idx, weight) with top-2 softmax."""
    n = flat_x.shape[0]
    logits = (flat_x @ Wr).astype(np.float32)
    ar = np.arange(n)
    i0 = logits.argmax(1)
    l0 = logits[ar, i0]
    masked = logits.copy()
    masked[ar, i0] = -np.inf
    i1 = masked.argmax(1)
    l1 = logits[ar, i1]
    # softmax over the two selected logits (l0 >= l1)
    e1 = np.exp((l1 - l0).astype(np.float32))
    w0 = 1.0 / (1.0 + e1)
    w1 = e1 / (1.0 + e1)
    experts = []
    for e in range(NUM_EXPERTS):
        m0 = i0 == e
        m1 = i1 == e
        idx = np.concatenate([ar[m0], ar[m1]])
        w = np.concatenate([w0[m0], w1[m1]]).astype(np.float32)
        experts.append((idx, w))
    return experts


def kernel(x, Wr, Wg, Wu, Wd):
    from concourse.bass_utils import run_bass_kernel_spmd

    B, T, C = x.shape
    x = np.asarray(x, dtype=np.float32)
    Wr = np.asarray(Wr, dtype=np.float32)
    Wg = np.asarray(Wg, dtype=np.float32)
    Wu = np.asarray(Wu, dtype=np.float32)
    Wd = np.asarray(Wd, dtype=np.float32)
    flat = x.reshape(-1, C)
    experts = _route(flat, Wr)

    n_max = max(len(idx) for idx, _ in experts)
    cap = max(TS, ((n_max + TS - 1) // TS) * TS)

    nc = _build_nc(cap)

    in_maps = []
    for e in range(NUM_EXPERTS):
        idx, _ = experts[e]
        xt = np.zeros((C, cap), dtype=np.float32)
        xt[:, : len(idx)] = flat[idx].T
        wg_t = np.ascontiguousarray(
            Wg[e].reshape(NC, P, NH, P).transpose(2, 1, 0, 3)
        )
        wu_t = np.ascontiguousarray(
            Wu[e].reshape(NC, P, NH, P).transpose(2, 1, 0, 3)
        )
        wd_t = np.ascontiguousarray(
            Wd[e].reshape(NH, P, NC, P).transpose(2, 1, 0, 3)
        )
        in_maps.append({"xt": xt, "wg_t": wg_t, "wu_t": wu_t, "wd_t": wd_t})

    res = run_bass_kernel_spmd(nc, in_maps, core_ids=list(range(8)))

    out = np.zeros((B * T, C), dtype=np.float64)
    for e in range(NUM_EXPERTS):
        idx, w = experts[e]
        r = res.results[e]
        ye = (r["yt0"].astype(np.float64) + r["yt1"]).T[: len(idx)]
        out[idx] += (w.astype(np.float64) * DEPTH_SCALE)[:, None] * ye
    return out.astype(np.float32).reshape(B, T, C)


if __name__ == "__main__":
    import reference

    inputs = reference.setup_inputs()
    out = kernel(**{k: np.asarray(v) for k, v in inputs.items()})
    print("kernel output", out.shape, out.dtype)
